# revision 24
# baseline (speedup 1.0000x reference)
"""Trainium2 kernel for nn_DeformableConvolution1D_60636348285726.

Problem structure (hardcoded): x [4,256,4096,1], offset/mod convs 256->5 with
kernel (5,1), main conv 256->256 kernel (5,1), stride 1, height pad 2,
width pad 1 (so output width is 3).

Key mathematical simplification (exact, holds for ANY input values):
  * The width-1 input is padded to width 3. Output width positions 0 and 2 of
    the offset/modulation convs sample only zero padding, so there
    dy = offset_b[k] and mask = sigmoid(mod_b[k]) -- constants per tap.
  * Bilinear sampling x-coords are 0,1,2 for the three output width
    positions. Valid x range is [0,0]: position 0 samples the real column
    with weight 1; positions 1 and 2 sample entirely out of range -> zero
    patches -> output planes 1,2 are exactly conv_b.
  * Therefore plane 0 is an ordinary dense 1D conv along T whose effective
    taps are built on the host from offset_b / mod_b / conv_w:
        for each k: tap (k + floor(ob_k))   gets s_k*(1-frac(ob_k))*conv_w[:,:,k]
                    tap (k + floor(ob_k)+1) gets s_k*frac(ob_k)    *conv_w[:,:,k]
    with s_k = sigmoid(mod_b[k]), sampling index h - 2 + tap, zero padded.
  * The offsets are ~N(0, 1e-4), so the two boundary taps (outside the
    5-tap window) carry ~1% of a single tap's coefficient. They are
    dropped when their combined Frobenius weight is < TRIM_TOL of the
    kernel (rel output error contribution ~ that fraction; tolerance is
    2e-2 and bf16 matmul noise is ~2e-3).

Device kernel: dense 1D conv [B=4, C=256, T=4096] -> [4, 256, 4096] with a
Ke-tap [256,256,Ke] effective kernel, run as PSUM-accumulated 128x128x512
bf16 matmuls (LDWEIGHTS double-buffers in bf16, so back-to-back matmuls
stream at N cycles). Sharding: 8 cores = 4 batches x 2 halves of T;
weights replicated.

The program is tuned for what the profiler actually counts
(first-useful-instruction .. end-of-teardown):
  * every `then_inc` site costs ~150ns of per-semaphore-register teardown
    on every engine, so the program uses ~18 events instead of 57;
  * input rides in 4 unchained DMAs (2 per HWDGE ring, the ring serializes
    them) so both rings stream at ~190GB/s with a single wait per pass;
  * junk matmuls warm the PE HAM (~3.6us) while the input DMA streams;
  * PSUM banks are evicted in 2-bank pairs f32->bf16, and the final
    output chunk is split across both rings to shorten the tail.
"""

import os
import numpy as np

# Problem constants (hardcoded per the task contract).
B, CIN, COUT, T, W = 4, 256, 256, 4096, 1
K, PAD = 5, 2
NCORES = 8
TC = T // 2          # per-core T span (B=4 x 2 halves = 8 shards)
NFREE = 512          # matmul moving free size / PSUM bank (f32 out limit)
P = 128              # partition dim
NJUNK = int(os.environ.get("DEFORM_NJUNK", "10"))  # HAM warm-up matmuls

# Matmul input dtype: "bf16" (default; LDWEIGHTS overlaps -> N cyc/matmul)
# or "f32r" (full-rate fp32, but LDWEIGHTS serializes -> N+128 cyc).
MM_DTYPE = os.environ.get("DEFORM_MM_DTYPE", "bf16")
# Drop boundary taps while their combined ||.||_F fraction stays below this.
TRIM_TOL = float(os.environ.get("DEFORM_TRIM_TOL", "8e-3"))
# DMA the last PSUM bank straight to DRAM (skips the final DVE hop).
LAST_PSUM = os.environ.get("DEFORM_LAST_PSUM", "1") == "1"

_PROGRAM_CACHE = {}


def _build_program(Ke: int, mm_dtype: str, last_psum: bool):
    """Build the per-core Bass program (identical on all 8 cores).

    Raw bass (no Tile): every cross-engine dependency is an explicit
    standalone wait instruction, because each TPB instruction encodes at
    most one semaphore wait.

    Per-core dataflow (4 chunk passes, 8 PSUM banks, one per (ch, co)):
      sync:   DMA xw[0:128] pieces (w+x0, x1, x2, x3) on ring SP; all 16
              DMA engines serve one dma_start at ~354GB/s and the two
              rings' FIFOs interleave, so pass-1-critical data leads both.
              Then out DMAs for even banks.
      scalar: same pieces for xw[128:256] on ring ACT + odd-bank outs;
              the final bank goes PSUM->DRAM f32 directly (skips the DVE
              hop; SBUF-source DMAs only trickle at ~48GB/s while the PE
              streams).
      tensor: junk warm-up MMs (HAM), then per chunk: gate on both rings'
              piece count, then 2co x (2ci x Ke) accumulated matmuls;
              each bank close bumps pe_sem.
      vector: memset warm region; evict each closed bank f32 -> bf16.
    """
    import concourse.bass as bass
    from concourse import mybir

    f32 = mybir.dt.float32
    mmdt = {"bf16": mybir.dt.bfloat16, "f32r": mybir.dt.float32r}[mm_dtype]
    odt = mybir.dt.bfloat16

    XL = TC + Ke - 1          # x slab columns
    WH = Ke * P               # weight columns per cout tile
    # Slab layout: [w_co0 (WH) | x (XL) | w_co1 (WH)] so the first DMA
    # piece gates only (chunk0, co0) and the PE starts ~2us earlier.
    XOFF = WH                 # x starts here
    WOFF = [0, WH + XL]       # per-co weight offsets
    SLAB = 2 * WH + XL
    nc = bass.Bass("TRN2", target_bir_lowering=False, debug=False)

    xw = nc.dram_tensor("xw", [CIN, SLAB], mmdt, kind="ExternalInput").ap()
    out = nc.dram_tensor("out", [COUT, TC], odt, kind="ExternalOutput").ap()

    NCH = TC // NFREE    # 4 chunks of 512
    # Input pieces in FIFO-need order: w_co0 + chunk0 halo, w_co1, then
    # the remaining x in disjoint 512-col extensions.
    pieces = [(0, XOFF + NFREE + Ke - 1), (WH + XL, SLAB)]
    for k in range(1, NCH):
        pieces.append((XOFF + k * NFREE + Ke - 1,
                       XOFF + (k + 1) * NFREE + Ke - 1))
    # Per-(ch, co) gate: how many pieces each ring must have delivered.
    gate = {(0, 0): 1, (0, 1): 2}
    for k in range(1, NCH):
        gate[(k, 0)] = gate[(k, 1)] = k + 2
    # banks in closure order: (ch, co) ch-major, co-inner
    banks = [(ch, co) for ch in range(NCH) for co in range(2)]
    NB = len(banks)

    with (
        nc.sbuf_tensor([P, SLAB], mmdt) as xw0,
        nc.sbuf_tensor([P, SLAB], mmdt) as xw1,
        nc.sbuf_tensor([P, TC], odt) as ot0,
        nc.sbuf_tensor([P, TC], odt) as ot1,
        nc.psum_tensor([P, NCH, NFREE], f32) as pt0,
        nc.psum_tensor([P, NCH, NFREE], f32) as pt1,
        nc.semaphore("inA_sem") as inA_sem,
        nc.semaphore("inB_sem") as inB_sem,
        nc.semaphore("pe_sem") as pe_sem,
        nc.semaphore("dve_sem") as dve_sem,
        nc.semaphore("out_sem") as out_sem,
        nc.semaphore("warm_sem") as warm_sem,
        nc.Block() as block,
    ):
        pts = [pt0, pt1]
        ots = [ot0, ot1]
        xw_sb = [xw0, xw1]

        def emit_out(eng, co, c0, c1, dve_count):
            # ot/out column range [c0, c1) of cout tile co, gated on the
            # DVE having evicted `dve_count` banks.
            eng.wait_ge(dve_sem, dve_count)
            eng.dma_start(
                out=out[co * P:(co + 1) * P, c0:c1],
                in_=ots[co][:, c0:c1],
            ).then_inc(out_sem, 16)

        @block.sync
        def _(sync):
            for c0, c1 in pieces:
                sync.dma_start(
                    out=xw0[:, c0:c1], in_=xw[0:P, c0:c1],
                ).then_inc(inA_sem, 16)
            emit_out(sync, 0, 0, 2 * NFREE, 3)          # banks 1,3
            emit_out(sync, 0, 2 * NFREE, 4 * NFREE, 7)  # banks 5,7
            # No final out_sem wait: the framework epilogue's per-engine
            # drain (gpsimd dma_reset over the kernel sem range) already
            # blocks until in-flight DMAs land, so the last transfers
            # overlap the fixed ~7us semaphore-reset teardown.

        @block.scalar
        def _(scalar):
            for c0, c1 in pieces:
                scalar.dma_start(
                    out=xw1[:, c0:c1], in_=xw[P:2 * P, c0:c1],
                ).then_inc(inB_sem, 16)
            emit_out(scalar, 1, 0, 2 * NFREE, 4)            # banks 2,4
            emit_out(scalar, 1, 2 * NFREE, 3 * NFREE, 6)    # bank 6
            emit_out(scalar, 1, 3 * NFREE, 4 * NFREE, 8)    # bank 8

        # Accumulation groups in closure order; the final bank is split in
        # two 256-col groups so the last evict+DMA tail is half-size.
        # (co, ch, sub0, sub1) with psum cols [sub0, sub1) inside the bank.
        groups = [(co, ch, 0, NFREE) for ch in range(NCH) for co in range(2)]

        @block.tensor
        def _(tensor):
            # HAM warm-up while the input DMAs run, sized so the PE goes
            # straight from junk to real matmuls with no idle gap (an idle
            # gap resets the HAM busy window and the real stream would
            # start at 1.2 GHz). Results are discarded (bank 0 re-opens
            # with start=True).
            tensor.wait_ge(warm_sem, 1)
            for _ in range(NJUNK):
                nc.tensor.matmul(
                    pts[0][:, 0, :],
                    lhsT=ot0[:, 0:P],
                    rhs=ot0[:, 0:NFREE],
                    start=True,
                    stop=True,
                )
            for co, ch, s0, s1 in groups:
                if s0 == 0 and (ch == 0 or co == 0):
                    g = gate[(ch, co)]
                    tensor.wait_ge(inA_sem, g * 16)
                    tensor.wait_ge(inB_sem, g * 16)
                for ci in range(2):
                    src = xw_sb[ci]
                    for j in range(Ke):
                        start = (ci == 0 and j == 0)
                        stop = (ci == 1 and j == Ke - 1)
                        mm = nc.tensor.matmul(
                            pts[co][:, ch, s0:s1],
                            lhsT=src[:, WOFF[co] + j * P:
                                     WOFF[co] + j * P + P],
                            rhs=src[:, XOFF + ch * NFREE + s0 + j:
                                    XOFF + ch * NFREE + s1 + j],
                            start=start,
                            stop=stop,
                        )
                        if stop:
                            mm.then_inc(pe_sem, 1)

        @block.vector
        def _(vector):
            nc.vector.memset(ot0[:, 0:NFREE], 0.0).then_inc(warm_sem, 1)
            # Evict each closed group PSUM f32 -> SBUF bf16.
            for k, (co, ch, s0, s1) in enumerate(groups):
                vector.wait_ge(pe_sem, k + 1)
                nc.vector.tensor_copy(
                    ots[co][:, ch * NFREE + s0:ch * NFREE + s1],
                    pts[co][:, ch, s0:s1],
                ).then_inc(dve_sem, 1)

    return nc


def _effective_taps(offset_b, mod_b, conv_w3):
    """Collapse offsets/modulation/conv_w into an effective conv kernel.

    Returns (E [COUT, CIN, Ke] f32, tmin) where plane-0 output is
    out0[b,o,h] = sum_{j,c} E[o,c,j] * xzero[b,c,h-PAD+tmin+j] + conv_b[o].

    Boundary taps are trimmed while their combined Frobenius fraction
    stays below TRIM_TOL (never below the 5 dominant taps).
    """
    ob = offset_b.astype(np.float64)
    f = np.floor(ob).astype(np.int64)
    w1 = ob - f
    w0 = 1.0 - w1
    s = 1.0 / (1.0 + np.exp(-mod_b.astype(np.float64)))

    tmin = int(min(k + f[k] for k in range(K)))
    tmax = int(max(k + f[k] + 1 for k in range(K)))
    Kf = tmax - tmin + 1
    E = np.zeros((COUT, CIN, Kf), np.float64)
    cw = conv_w3.astype(np.float64)
    for k in range(K):
        E[:, :, k + f[k] - tmin] += cw[:, :, k] * (s[k] * w0[k])
        E[:, :, k + f[k] + 1 - tmin] += cw[:, :, k] * (s[k] * w1[k])

    # Trim low-weight boundary taps (greedy, smaller edge first).
    norms2 = np.einsum('ocj,ocj->j', E, E)
    total = float(np.sqrt(norms2.sum()))
    lo, hi = 0, Kf  # active window [lo, hi)
    dropped2 = 0.0
    while hi - lo > K:
        edge = lo if norms2[lo] <= norms2[hi - 1] else hi - 1
        nd2 = dropped2 + float(norms2[edge])
        if np.sqrt(nd2) / total > TRIM_TOL:
            break
        dropped2 = nd2
        if edge == lo:
            lo += 1
        else:
            hi -= 1
    E = E[:, :, lo:hi]
    return np.ascontiguousarray(E).astype(np.float32), tmin + lo


def _run(inputs, trace=False, tmpdir=None):
    from concourse.bass_utils import run_bass_kernel_spmd
    import ml_dtypes

    x = np.asarray(inputs["x"], np.float32)
    offset_b = np.asarray(inputs["offset_b"], np.float32)
    mod_b = np.asarray(inputs["mod_b"], np.float32)
    conv_w = np.asarray(inputs["conv_w"], np.float32)
    conv_b = np.asarray(inputs["conv_b"], np.float32)
    assert x.shape == (B, CIN, T, W), x.shape

    x3 = np.ascontiguousarray(x[:, :, :, 0])            # [B,C,T]
    conv_w3 = np.ascontiguousarray(conv_w[:, :, :, 0])  # [O,C,K]

    E, tmin = _effective_taps(offset_b, mod_b, conv_w3)
    Ke = E.shape[2]

    # Zero-padded x so that per-core slabs are uniform:
    # xp[:, :, i] = x[:, :, i - L] (zero outside), L = PAD - tmin.
    L = PAD - tmin
    Tp = T + Ke - 1
    xp = np.zeros((B, CIN, Tp), np.float32)
    lo, hi = max(0, L), min(Tp, L + T)
    if lo < hi:
        xp[:, :, lo:hi] = x3[:, :, lo - L:hi - L]

    # Weights in per-co lhsT layout: wt[co, ci, j*P + p] = E[co*P + p, ci, j].
    wt = np.ascontiguousarray(
        E.reshape(2, P, CIN, Ke).transpose(0, 2, 3, 1).reshape(2, CIN, Ke * P))

    np_dt = np.float32
    if MM_DTYPE == "bf16":
        np_dt = ml_dtypes.bfloat16
        xp = xp.astype(np_dt)
        wt = wt.astype(np_dt)

    key = (Ke, MM_DTYPE, LAST_PSUM)
    if key not in _PROGRAM_CACHE:
        _PROGRAM_CACHE[key] = _build_program(Ke, MM_DTYPE, LAST_PSUM)
    nc = _PROGRAM_CACHE[key]

    XL = TC + Ke - 1
    WH = Ke * P
    in_maps = []
    for core in range(NCORES):
        b, half = core // 2, core % 2
        t0 = half * TC
        # Slab layout: [w_co0 | x | w_co1] (matches the device program).
        xwm = np.empty((CIN, 2 * WH + XL), np_dt)
        xwm[:, :WH] = wt[0]
        xwm[:, WH:WH + XL] = xp[b, :, t0: t0 + XL]
        xwm[:, WH + XL:] = wt[1]
        in_maps.append({"xw": xwm})

    res = run_bass_kernel_spmd(
        nc, in_maps, core_ids=list(range(NCORES)),
        trace=trace, tmpdir=tmpdir,
    )

    out = np.empty((B, COUT, T, 3), np.float32)
    out[:, :, :, 1] = conv_b[None, :, None]
    out[:, :, :, 2] = conv_b[None, :, None]
    for core in range(NCORES):
        b, half = core // 2, core % 2
        out[b, :, half * TC:(half + 1) * TC, 0] = (
            res.results[core]["out"].astype(np.float32))
    out[:, :, :, 0] += conv_b[None, :, None]
    return out, res


def kernel(**inputs):
    out, _ = _run(inputs, trace=False)
    return out
